# revision 62
# baseline (speedup 1.0000x reference)
"""Bootstrap-ensemble MLP (100 models, D=16 -> H=128 x5 -> mu/sigma heads)
on 8 Trainium2 NeuronCores.

Sharding: every core runs an identical SPMD program over 25 models x 8192
batch points (model axis split 4 ways x batch split 2 ways) -- perfectly
balanced.  All per-core weights are pre-arranged on the host into the exact
SBUF layouts the TensorEngine wants (lhsT = pre-transposed stationary
operand), so the device does no transposes at all.

Compute structure per core:
- bf16 matmul operands (fp32 PSUM accumulation), biases fp32
- models interleaved in 5 groups of 5 so PE always has independent matmuls
  while ACT/DVE run another model's bias+ReLU (fused into one op each); a
  4+1 grouping leaves a 1-model straggler unit whose production valley
  starves ACT/DVE at every chunk edge (measured -22us when removed)
- layer-1 (K=17, bias folded in as an extra contraction row against a
  constant-one row of x) matmuls run pairwise-concurrent in different
  32-row quadrants of the PE array (tile_position row tiling; quadrant =
  model index mod 4, matching the host w1t packing)
- 4 rotating [128, CH] PSUM tiles (full 8 banks): the mu/sigma head matmuls
  run as a deferred per-chunk streak (from saved layer-4 h tiles) into a
  transiently-held pool tile, col-tiled 2x so even/odd models' head matmuls
  run concurrently in different column halves of the PE array
- the streak is emitted in 6 segments with the next two units' L1 matmul
  pairs interleaved between them (their relus DVE-biased), so the PE keeps
  producing relu-able psum tiles mid-streak and ACT/DVE never fully starve
  during the ~5us head-only window; the final chunk's heads interleave
  into the last unit so only the last group's heads remain as tail
  (combined measured: -24us total wall)
- a post-schedule pass deletes LDWEIGHTS instructions whose exact weights
  are already resident in the targeted PE-array region (the Tile lowering
  re-emits one per matmul; weights persist across matmuls)
- bias+ReLU ops are assigned to ACT vs DVE by a compile-time soonest-
  finish rule: min over engines of max(engine_free, modeled_mm_done) +
  op_cost, using cost models fit from HW profiles and a modeled PE clock

Performance note (profiled on HW): this kernel is activation-engine bound.
Every matmul output column must cross PSUM->SBUF through exactly one
ACT/DVE op (ReLU fused with the bias add); on TRN2 those are the only two
engines with a PSUM port (GpSimd/DMA have none, 16-bit PSUM matmul output
is TRN3-only -- the walrus verifier rejects it), and each reads fp32 PSUM
at 1 elem/lane/cycle.  That puts a hard ~613us/engine floor on the 1.02M
drain columns per core; measured ACT/DVE busy is ~598-616us (97% of floor)
with all three engines balanced within 5%.  Restructuring experiments
(staged epilogues, bank-major head streaks with mid-streak epilogue
overlap, soonest-start relu assignment, bigger/smaller PSUM tiles) all
landed within +-1.5us of this schedule; overlapping engine PSUM reads with
the in-flight head-streak accumulation in the sibling bank was measurably
race-prone on HW (nondeterministic corruption) and must be avoided.

Two further dead ends, verified by experiment: (1) fusing the two bias-free
L1 ReLUs of a pair into one FD=2048 op requires address-adjacent PSUM slots,
but neither a raw nc.alloc_psum_tensor nor a single full-PSUM pool tile
carved into manual slots is safe -- the Tile framework enforces
write-after-read ordering between successive USES of a PSUM region through
the pool allocation gate, not through subtile AP overlap, so manual slot
aliasing loses WAR deps and corrupts (identical failure both ways).
(2) Sustained back-to-back benching drives the chip into P0 downclock
(~2.0 GHz, everything uniformly ~1.2x slower); let it cool before judging
a measurement.
"""

import os

import numpy as np

M = 100  # n_models
D = 16  # input_dim
H = 128  # hidden_dim
O = 1  # output_dim
NH = 4  # n_hidden
N = 16384  # batch of query points

NCORES = 8
MPC = 25  # models per core
NB = 4  # model blocks
NHALF = N // 2  # 8192 points per core
CH = 1024  # chunk of batch points processed at once
NCH = NHALF // CH  # 8 chunks
MM_N = 512  # matmul moving free dim (one PSUM bank of fp32)
NEV = (MPC + 1) // 2  # 13 even-index models (head col-group 0)
NOD = MPC // 2  # 12 odd-index models (head col-group 1)

_CACHE: dict = {}


def _build_module():
    import concourse.bacc as bacc
    import concourse.mybir as mybir
    import concourse.tile as tile

    f32 = mybir.dt.float32
    mmdt = os.environ.get("KERNEL_MM_DTYPE", "bf16")
    f32m = {
        "bf16": mybir.dt.bfloat16,
        "fp16": mybir.dt.float16,
        "fp32r": mybir.dt.float32r,
        "fp32": mybir.dt.float32,
    }[mmdt]
    AF = mybir.ActivationFunctionType
    ALU = mybir.AluOpType

    nc = bacc.Bacc(
        "TRN2",
        target_bir_lowering=False,
        debug=False,
        num_devices=NCORES,
    )

    NBLK = (MPC + 3) // 4  # 7 row-tiling blocks of up to 4 models
    DK = D + 1  # L1 contraction rows incl folded bias
    xt_d = nc.dram_tensor("xt", [128, NHALF], f32m, kind="ExternalInput")
    w1t_d = nc.dram_tensor("w1t", [128, NBLK * H], f32m, kind="ExternalInput")
    wht_d = nc.dram_tensor("wht", [H, MPC * NH * H], f32m, kind="ExternalInput")
    whd_d = nc.dram_tensor("whd", [H, MPC * 64], f32m, kind="ExternalInput")
    bh_d = nc.dram_tensor("bh", [H, MPC * NH], f32, kind="ExternalInput")
    bhd_d = nc.dram_tensor("bhd", [128, 1], f32, kind="ExternalInput")
    mu_d = nc.dram_tensor("mu", [MPC, NHALF], f32, kind="ExternalOutput")
    sig_d = nc.dram_tensor("sig", [MPC, NHALF], f32, kind="ExternalOutput")

    # compile-time engine load balancing (ns, cost models fit from HW
    # profiles).  pe_clock models the PE timeline so relu assignment can
    # pick the engine that STARTS each drain soonest -- min over engines of
    # max(engine_free, mm_done) -- instead of pure aggregate-load balance,
    # cutting per-op MM-wait micro-gaps.
    eng_load = {"act": 0.0, "dve": 0.0}
    pe_state = {"t": 0.0}

    def pe_advance(cycles):
        # 2.4 GHz warm PE; cycles = moving columns + per-MM overhead
        pe_state["t"] += cycles / 2.4

    with tile.TileContext(nc) as tc:
        with (
            tc.tile_pool(name="const", bufs=1) as const,
            tc.tile_pool(name="hpool", bufs=52) as hpool,
            tc.tile_pool(name="opool", bufs=4) as opool,
            tc.tile_pool(name="mmpsum", bufs=4, space="PSUM") as mmpsum,
        ):
            xt = const.tile([128, NHALF], f32m)
            w1t = const.tile([128, NBLK * H], f32m)
            wht = const.tile([H, MPC * NH * H], f32m)
            whd = const.tile([H, MPC * 64], f32m)
            bh = const.tile([H, MPC * NH], f32)
            bhd = const.tile([128, 1], f32)

            # ordered so the first unit's operands land first: w1t + x
            # chunk 0 unblock layer-1 within a few microseconds, hidden
            # weights stream in per-model ahead of use, head weights last
            def dma_xt(nt):
                s = nt * CH
                nc.sync.dma_start(xt[:, s : s + CH], xt_d[:, s : s + CH])

            def dma_wht(m):
                s = m * NH * H
                nc.sync.dma_start(wht[:, s : s + NH * H], wht_d[:, s : s + NH * H])

            # group-0 L1 weights + first 512 x columns land first so the
            # very first matmul unblocks as early as possible
            nc.sync.dma_start(w1t[:, 0:H], w1t_d[:, 0:H])
            nc.sync.dma_start(xt[:, 0:MM_N], xt_d[:, 0:MM_N])
            nc.sync.dma_start(xt[:, MM_N:CH], xt_d[:, MM_N:CH])
            nc.sync.dma_start(w1t[:, H:], w1t_d[:, H:])
            nc.sync.dma_start(bh[:], bh_d[:])
            for m in range(4):
                dma_wht(m)
            dma_xt(1)
            for m in range(4, 12):
                dma_wht(m)
            dma_xt(2)
            dma_xt(3)
            for m in range(12, MPC):
                dma_wht(m)
            for nt in range(4, NCH):
                dma_xt(nt)
            nc.sync.dma_start(whd[:], whd_d[:])
            nc.sync.dma_start(bhd[:], bhd_d[:])

            def relu(dst, src, bias_ap, fd, force=None):
                # pick the engine that would FINISH this drain soonest given
                # both its queued load and the source MM's modeled completion
                # (constants fit from HW profile: ACT=(FD+311)/1.2,
                #  DVE=(FD+207)/0.96)
                ready = pe_state["t"]
                f_act = max(eng_load["act"], ready) + (fd + 311) / 1.2
                f_dve = max(eng_load["dve"], ready) + (fd + 207) / 0.96
                pick_act = f_act <= f_dve
                if force is not None:
                    pick_act = force == "act"
                if pick_act:
                    eng_load["act"] = f_act
                    if bias_ap is None:
                        nc.scalar.activation(dst, src, AF.Relu)
                    else:
                        nc.scalar.activation(dst, src, AF.Relu, bias=bias_ap)
                else:
                    eng_load["dve"] = f_dve
                    if bias_ap is None:
                        nc.vector.tensor_scalar_max(dst, src, 0.0)
                    else:
                        nc.vector.tensor_scalar(
                            dst, src, bias_ap, 0.0, ALU.add, ALU.max
                        )

            # hidden-pipeline groups of 5 (5x5=25, no 1-model straggler
            # group -- a 1-model unit is a PE production valley that starves
            # ACT/DVE at every chunk edge); L1 row-quadrants stay per-model
            groups = [list(range(b * 5, b * 5 + 5)) for b in range(5)]
            units = [(nt, bi) for nt in range(NCH) for bi in range(len(groups))]
            h_l1 = {}
            h_fin = {}  # (nt, m) -> final-layer h tile awaiting head matmuls

            def emit_l1_pair(nt, bi, p0, dve_bias=False):
                # 2-model row-tiled pair: each model gets its own contiguous
                # [128, CH] psum tile so its ReLU runs as one FD=CH op
                c0 = nt * CH
                grp = groups[bi]
                pair = grp[p0 : p0 + 2]
                for m in pair:
                    h_l1[(nt, m)] = hpool.tile([128, CH], f32m, tag="h", name="h")
                tiles = [
                    mmpsum.tile([128, CH], f32, tag="mm", name="l1ps")
                    for _ in pair
                ]
                for s in range(0, CH, MM_N):
                    for k, m in enumerate(pair):
                        # per-model w1t block/quadrant (host packs model m at
                        # block m//4, row-quadrant m%4); consecutive models
                        # always land in distinct quadrants -> pair overlaps
                        b, g = m // 4, m % 4
                        nc.tensor.matmul(
                            tiles[k][:, s : s + MM_N],
                            w1t[32 * g : 32 * g + DK, b * H : (b + 1) * H],
                            xt[32 * g : 32 * g + DK, c0 + s : c0 + s + MM_N],
                            start=True,
                            stop=True,
                            tile_position=(32 * g, 0),
                        )
                        pe_advance(MM_N / 2 + 64)
                for k, m in enumerate(pair):
                    # during a head streak DVE is the starving engine: pin
                    # the second relu of each mid-streak L1 pair to it
                    force = "dve" if (dve_bias and k == 1) else None
                    relu(h_l1[(nt, m)][:], tiles[k][:], None, CH, force=force)

            head_state = {}

            def emit_head_model(nt, m):
                """Head matmuls for one model, into the chunk's shared hp
                tile: even models stream through array cols 0-63 (psum
                partitions 0-63), odd through cols 64-127 (partitions
                64-127).  Accumulation groups interleave safely: HW
                has_written clearing is per written region (verified)."""
                if "hp" not in head_state:
                    head_state["hp"] = mmpsum.tile([128, CH], f32, tag="mm", name="hp")
                hp = head_state["hp"]
                g = m % 2
                lhshd = whd[:, m * 64 : (m + 1) * 64]
                hf = h_fin.pop((nt, m))
                for s in range(0, CH, MM_N):
                    nc.tensor.matmul(
                        hp[64 * g : 64 * g + 64, s : s + MM_N],
                        lhshd,
                        hf[:, s : s + MM_N],
                        start=(m <= 1),
                        stop=(m >= MPC - 2),
                        tile_position=(0, 64 * g),
                        skip_group_check=True,
                    )
                    pe_advance(MM_N / 2 + 50)

            def emit_head_epilogue(nt):
                """mu (DVE bias-add) / sigma (ACT exp) + DMA out."""
                c0 = nt * CH
                hp = head_state.pop("hp")
                mu_t = opool.tile([128, CH], f32, tag="mu")
                sig_t = opool.tile([128, CH], f32, tag="sig")
                # each op's DMA issues right after it (not batched at the
                # end) so outputs drain and opool tiles free ~1-2us earlier
                nc.vector.tensor_scalar_add(
                    mu_t[0:NEV, :], hp[0:NEV, :], bhd[0:NEV, :]
                )
                nc.sync.dma_start(mu_d[0:NEV, c0 : c0 + CH], mu_t[0:NEV, :])
                nc.scalar.activation(
                    sig_t[32 : 32 + NEV, :], hp[32 : 32 + NEV, :], AF.Exp,
                    bias=bhd[32 : 32 + NEV, :],
                )
                nc.sync.dma_start(
                    sig_d[0:NEV, c0 : c0 + CH], sig_t[32 : 32 + NEV, :]
                )
                nc.vector.tensor_scalar_add(
                    mu_t[64 : 64 + NOD, :], hp[64 : 64 + NOD, :],
                    bhd[64 : 64 + NOD, :],
                )
                nc.sync.dma_start(
                    mu_d[NEV:MPC, c0 : c0 + CH], mu_t[64 : 64 + NOD, :]
                )
                eng_load["dve"] += 2 * (CH + 207) / 0.96
                nc.scalar.activation(
                    sig_t[96 : 96 + NOD, :], hp[96 : 96 + NOD, :], AF.Exp,
                    bias=bhd[96 : 96 + NOD, :],
                )
                nc.sync.dma_start(
                    sig_d[NEV:MPC, c0 : c0 + CH], sig_t[96 : 96 + NOD, :]
                )
                eng_load["act"] += 2 * (CH + 311) / 1.2

            l1_emitted = set()

            def maybe_emit_l1_pair(uidx, p0, dve_bias=False):
                if uidx < len(units) and (uidx, p0) not in l1_emitted:
                    nt2, bi2 = units[uidx]
                    if p0 < len(groups[bi2]):
                        l1_emitted.add((uidx, p0))
                        emit_l1_pair(nt2, bi2, p0, dve_bias=dve_bias)

            def maybe_emit_l1(uidx):
                if uidx < len(units):
                    for p0 in range(0, len(groups[units[uidx][1]]), 2):
                        maybe_emit_l1_pair(uidx, p0)

            maybe_emit_l1(0)
            LAST = NCH - 1

            def emit_head_streak(ntq, l1_uidx=None):
                # Interleave the next unit's two L1 matmul pairs into the
                # streak so the PE keeps producing relu-able psum tiles
                # mid-streak: each engine dry-spell shrinks to roughly the
                # backlog the engines can cover.  Epilogue stays strictly
                # after ALL head MMs (overlapped PSUM reads of the hp tile
                # were measured race-prone on HW -- do not reorder).
                # two feeds up front build an engine-work cushion (engines
                # consume feed supply ~1us/segment faster than the PE can
                # deliver it when evenly spaced -> observed mid-streak waits)
                segs = [(0, 5), (5, 10), (10, 15), (15, 20), (20, MPC)]
                l1_feed = [(0, 4), (1, 0), (1, 2), (1, 4), None]
                if l1_uidx is not None:
                    maybe_emit_l1_pair(l1_uidx, 0, dve_bias=True)
                    maybe_emit_l1_pair(l1_uidx, 2, dve_bias=True)
                for (m0, m1), fd in zip(segs, l1_feed):
                    if l1_uidx is not None and fd is not None:
                        maybe_emit_l1_pair(l1_uidx + fd[0], fd[1], dve_bias=True)
                    for mq in range(m0, m1):
                        emit_head_model(ntq, mq)
                emit_head_epilogue(ntq)

            for u, (nt, bi) in enumerate(units):
                grp = groups[bi]
                last_unit = u == len(units) - 1
                if last_unit:
                    # the final chunk's heads for models finished by earlier
                    # units run interleaved with this unit so only the last
                    # group's heads + epilogue remain as tail
                    for mq in range(0, 12):
                        emit_head_model(LAST, mq)
                hcur = {m: h_l1.pop((nt, m)) for m in grp}
                # hidden layers, interleaved across the group
                for i in range(NH):
                    for m in grp:
                        ps = mmpsum.tile([128, CH], f32, tag="mm")
                        lhsh = wht[:, (m * NH + i) * H : (m * NH + i + 1) * H]
                        for s in range(0, CH, MM_N):
                            nc.tensor.matmul(
                                ps[:, s : s + MM_N],
                                lhsh,
                                hcur[m][:, s : s + MM_N],
                                start=True,
                                stop=True,
                            )
                            pe_advance(MM_N + 6)
                        hn = hpool.tile([128, CH], f32m, tag="h")
                        bias_ap = bh[:, m * NH + i : m * NH + i + 1]
                        relu(hn[:], ps[:], bias_ap, CH)
                        hcur[m] = hn
                    if i == NH - 2:
                        if bi == 2 and nt >= 1:
                            # previous chunk's head streak with the next
                            # unit's L1 pairs interleaved mid-streak so
                            # ACT/DVE keep getting fresh relu work
                            emit_head_streak(nt - 1, l1_uidx=u + 1)
                        else:
                            # prefetch the next unit's layer-1
                            maybe_emit_l1(u + 1)
                    if last_unit and i == 1:
                        for mq in range(12, 20):
                            emit_head_model(LAST, mq)
                for m in grp:
                    h_fin[(nt, m)] = hcur[m]
            for mq in range(20, MPC):
                emit_head_model(LAST, mq)
            emit_head_epilogue(LAST)

    _dedupe_ldweights(nc, mybir)
    nc.compile()
    return nc


def _dedupe_ldweights(nc, mybir):
    """Delete LDWEIGHTS whose exact weights are already resident in the same
    PE-array region (the Tile lowering re-emits one per matmul).  Weights
    persist in the array across matmuls, so back-to-back matmuls on the same
    stationary operand only need the first load.  Region tracking handles
    row/col-tiled partial loads (an overlapping load invalidates)."""
    removed = 0
    for blk in nc.main_func.blocks:
        loaded: dict = {}
        out = []
        for inst in blk.instructions:
            if isinstance(inst, mybir.InstLdweights):
                w = inst.ins[0]
                key = (
                    getattr(w, "memref", None),
                    w.offset,
                    str(w.ap),
                    str(w.dtype),
                    inst.tile_position,
                    inst.tile_size,
                    inst.perf_mode,
                    inst.is_transpose,
                )
                tp = inst.tile_position or (0, 0)
                ts = inst.tile_size or (128, 128)
                region = (tp[0], tp[0] + ts[0], tp[1], tp[1] + ts[1])
                si = inst.sync_info
                no_sync = si is None or (not si.on_wait and not si.on_update)
                if no_sync and loaded.get(region) == key:
                    removed += 1
                    continue
                for r in list(loaded):
                    if not (
                        r[1] <= region[0]
                        or region[1] <= r[0]
                        or r[3] <= region[2]
                        or region[3] <= r[2]
                    ):
                        del loaded[r]
                loaded[region] = key
            out.append(inst)
        blk.instructions[:] = out
    return removed


def _get_module():
    if "nc" not in _CACHE:
        _CACHE["nc"] = _build_module()
    return _CACHE["nc"]


def _mm_np_dtype():
    mmdt = os.environ.get("KERNEL_MM_DTYPE", "bf16")
    if mmdt == "bf16":
        import ml_dtypes

        return ml_dtypes.bfloat16
    if mmdt == "fp16":
        return np.float16
    return np.float32


def _shard_inputs(x, W1, b1, Wh, bh, Wmu, bmu, Wsig, bsig):
    """Build the per-core input maps (host-side layout prep)."""
    NBLK = (MPC + 3) // 4
    mdt = _mm_np_dtype()
    in_maps = []
    for c in range(NCORES):
        mb, half = c % NB, c // NB
        ms = slice(MPC * mb, MPC * (mb + 1))
        xh = x[NHALF * half : NHALF * (half + 1), :]  # [8192, 16]
        xtr = np.ascontiguousarray(xh.T)  # [16, 8192]
        xt_full = np.zeros((128, NHALF), dtype=np.float32)
        for rep in range(4):  # replicas at partition 0/32/64/96 for row tiling
            xt_full[32 * rep : 32 * rep + D, :] = xtr
            xt_full[32 * rep + D, :] = 1.0  # constant row for folded L1 bias

        w1 = W1[ms]  # [25, 128, 16]
        b1c = b1[ms]  # [25, 128]
        w1t = np.zeros((128, NBLK * H), dtype=np.float32)
        for m in range(MPC):
            b, g = m // 4, m % 4
            w1t[32 * g : 32 * g + D, b * H : (b + 1) * H] = w1[m].T
            w1t[32 * g + D, b * H : (b + 1) * H] = b1c[m]  # folded bias row

        wh = Wh[ms]  # [25, 4, 128, 128] (out, in)
        wht = np.ascontiguousarray(
            wh.transpose(3, 0, 1, 2).reshape(H, MPC * NH * H)
        )  # [h_in, (m, i, h_out)]

        # head weights, col-tiled 2x: even models -> array cols 0-63, odd ->
        # cols 64-127; within the 64-col block: col k=m//2 = Wmu, 32+k = Wsig
        whd = np.zeros((H, MPC * 64), dtype=np.float32)
        for m in range(MPC):
            base, k = m * 64, m // 2
            whd[:, base + k] = Wmu[ms][m, 0, :]
            whd[:, base + 32 + k] = Wsig[ms][m, 0, :]

        bhp = np.ascontiguousarray(
            bh[ms].transpose(2, 0, 1).reshape(H, MPC * NH)
        )  # [128, (m, i)]
        bhdp = np.zeros((128, 1), dtype=np.float32)
        bhdp[0:NEV, 0] = bmu[ms][0::2, 0]
        bhdp[32 : 32 + NEV, 0] = bsig[ms][0::2, 0]
        bhdp[64 : 64 + NOD, 0] = bmu[ms][1::2, 0]
        bhdp[96 : 96 + NOD, 0] = bsig[ms][1::2, 0]

        in_maps.append(
            {
                "xt": xt_full.astype(mdt),
                "w1t": w1t.astype(mdt),
                "wht": wht.astype(mdt),
                "whd": whd.astype(mdt),
                "bh": bhp,
                "bhd": bhdp,
            }
        )
    return in_maps


def _run(in_maps, trace=False):
    from concourse.bass_utils import run_bass_kernel_spmd

    nc = _get_module()
    return run_bass_kernel_spmd(
        nc, in_maps, list(range(NCORES)), trace=trace
    )


def kernel(x, W1, b1, Wh, bh, Wmu, bmu, Wsig, bsig):
    args = [
        np.ascontiguousarray(np.asarray(a, dtype=np.float32))
        for a in (x, W1, b1, Wh, bh, Wmu, bmu, Wsig, bsig)
    ]
    in_maps = _shard_inputs(*args)
    res = _run(in_maps, trace=bool(int(os.environ.get("KERNEL_TRACE", "0"))))
    _CACHE["last_results"] = res

    mu = np.empty((M, N), dtype=np.float32)
    sig = np.empty((M, N), dtype=np.float32)
    ev = np.arange(0, MPC, 2)  # storage rows 0..12 hold even models
    od = np.arange(1, MPC, 2)  # storage rows 13..24 hold odd models
    for c in range(NCORES):
        mb, half = c % NB, c // NB
        m0 = MPC * mb
        ns = slice(NHALF * half, NHALF * (half + 1))
        r = res.results[c]
        mu[m0 + ev, ns] = r["mu"][0:NEV]
        mu[m0 + od, ns] = r["mu"][NEV:MPC]
        sig[m0 + ev, ns] = r["sig"][0:NEV]
        sig[m0 + od, ns] = r["sig"][NEV:MPC]
    return (mu.reshape(M, N, O), sig.reshape(M, N, O))


# revision 63
# speedup vs baseline: 1.0112x; 1.0112x over previous
"""Bootstrap-ensemble MLP (100 models, D=16 -> H=128 x5 -> mu/sigma heads)
on 8 Trainium2 NeuronCores.

Sharding: every core runs an identical SPMD program over 25 models x 8192
batch points (model axis split 4 ways x batch split 2 ways) -- perfectly
balanced.  All per-core weights are pre-arranged on the host into the exact
SBUF layouts the TensorEngine wants (lhsT = pre-transposed stationary
operand), so the device does no transposes at all.

Compute structure per core:
- bf16 matmul operands (fp32 PSUM accumulation), biases fp32
- models interleaved in 5 groups of 5 so PE always has independent matmuls
  while ACT/DVE run another model's bias+ReLU (fused into one op each); a
  4+1 grouping leaves a 1-model straggler unit whose production valley
  starves ACT/DVE at every chunk edge (measured -22us when removed)
- layer-1 (K=17, bias folded in as an extra contraction row against a
  constant-one row of x) matmuls run pairwise-concurrent in different
  32-row quadrants of the PE array (tile_position row tiling; quadrant =
  model index mod 4, matching the host w1t packing)
- 4 rotating [128, CH] PSUM tiles (full 8 banks): the mu/sigma head matmuls
  run as a deferred per-chunk streak (from saved layer-4 h tiles) into a
  transiently-held pool tile, col-tiled 2x so even/odd models' head matmuls
  run concurrently in different column halves of the PE array
- the streak is emitted in 6 segments with the next two units' L1 matmul
  pairs interleaved between them (their relus DVE-biased), so the PE keeps
  producing relu-able psum tiles mid-streak and ACT/DVE never fully starve
  during the ~5us head-only window; the final chunk's heads interleave
  into the last unit so only the last group's heads remain as tail
  (combined measured: -24us total wall)
- a post-schedule pass deletes LDWEIGHTS instructions whose exact weights
  are already resident in the targeted PE-array region (the Tile lowering
  re-emits one per matmul; weights persist across matmuls)
- bias+ReLU ops are assigned to ACT vs DVE by a compile-time soonest-
  finish rule: min over engines of max(engine_free, modeled_mm_done) +
  op_cost, using cost models fit from HW profiles and a modeled PE clock

Performance note (profiled on HW): this kernel is activation-engine bound.
Every matmul output column must cross PSUM->SBUF through exactly one
ACT/DVE op (ReLU fused with the bias add); on TRN2 those are the only two
engines with a PSUM port (GpSimd/DMA have none, 16-bit PSUM matmul output
is TRN3-only -- the walrus verifier rejects it), and each reads fp32 PSUM
at 1 elem/lane/cycle.  That puts a hard ~613us/engine floor on the 1.02M
drain columns per core; measured ACT/DVE busy is ~598-616us (97% of floor)
with all three engines balanced within 5%.  Restructuring experiments
(staged epilogues, bank-major head streaks with mid-streak epilogue
overlap, soonest-start relu assignment, bigger/smaller PSUM tiles) all
landed within +-1.5us of this schedule; overlapping engine PSUM reads with
the in-flight head-streak accumulation in the sibling bank was measurably
race-prone on HW (nondeterministic corruption) and must be avoided.

Two further dead ends, verified by experiment: (1) fusing the two bias-free
L1 ReLUs of a pair into one FD=2048 op requires address-adjacent PSUM slots,
but neither a raw nc.alloc_psum_tensor nor a single full-PSUM pool tile
carved into manual slots is safe -- the Tile framework enforces
write-after-read ordering between successive USES of a PSUM region through
the pool allocation gate, not through subtile AP overlap, so manual slot
aliasing loses WAR deps and corrupts (identical failure both ways).
(2) Sustained back-to-back benching drives the chip into P0 downclock
(~2.0 GHz, everything uniformly ~1.2x slower); let it cool before judging
a measurement.
"""

import os

import numpy as np

M = 100  # n_models
D = 16  # input_dim
H = 128  # hidden_dim
O = 1  # output_dim
NH = 4  # n_hidden
N = 16384  # batch of query points

NCORES = 8
MPC = 25  # models per core
NB = 4  # model blocks
NHALF = N // 2  # 8192 points per core
CH = 1024  # chunk of batch points processed at once
NCH = NHALF // CH  # 8 chunks
MM_N = 512  # matmul moving free dim (one PSUM bank of fp32)
NEV = (MPC + 1) // 2  # 13 even-index models (head col-group 0)
NOD = MPC // 2  # 12 odd-index models (head col-group 1)

_CACHE: dict = {}


def _build_module():
    import concourse.bacc as bacc
    import concourse.mybir as mybir
    import concourse.tile as tile

    f32 = mybir.dt.float32
    mmdt = os.environ.get("KERNEL_MM_DTYPE", "bf16")
    f32m = {
        "bf16": mybir.dt.bfloat16,
        "fp16": mybir.dt.float16,
        "fp32r": mybir.dt.float32r,
        "fp32": mybir.dt.float32,
    }[mmdt]
    AF = mybir.ActivationFunctionType
    ALU = mybir.AluOpType

    nc = bacc.Bacc(
        "TRN2",
        target_bir_lowering=False,
        debug=False,
        num_devices=NCORES,
    )

    NBLK = (MPC + 3) // 4  # 7 row-tiling blocks of up to 4 models
    DK = D + 1  # L1 contraction rows incl folded bias
    xt_d = nc.dram_tensor("xt", [128, NHALF], f32m, kind="ExternalInput")
    w1t_d = nc.dram_tensor("w1t", [128, NBLK * H], f32m, kind="ExternalInput")
    wht_d = nc.dram_tensor("wht", [H, MPC * NH * H], f32m, kind="ExternalInput")
    whd_d = nc.dram_tensor("whd", [H, MPC * 64], f32m, kind="ExternalInput")
    bh_d = nc.dram_tensor("bh", [H, MPC * NH], f32, kind="ExternalInput")
    bhd_d = nc.dram_tensor("bhd", [128, 1], f32, kind="ExternalInput")
    mu_d = nc.dram_tensor("mu", [MPC, NHALF], f32, kind="ExternalOutput")
    sig_d = nc.dram_tensor("sig", [MPC, NHALF], f32, kind="ExternalOutput")

    # compile-time engine load balancing (ns, cost models fit from HW
    # profiles).  pe_clock models the PE timeline so relu assignment can
    # pick the engine that STARTS each drain soonest -- min over engines of
    # max(engine_free, mm_done) -- instead of pure aggregate-load balance,
    # cutting per-op MM-wait micro-gaps.
    eng_load = {"act": 0.0, "dve": 0.0}
    pe_state = {"t": 0.0}

    def pe_advance(cycles):
        # 2.4 GHz warm PE; cycles = moving columns + per-MM overhead
        pe_state["t"] += cycles / 2.4

    with tile.TileContext(nc) as tc:
        with (
            tc.tile_pool(name="const", bufs=1) as const,
            tc.tile_pool(name="hpool", bufs=52) as hpool,
            tc.tile_pool(name="opool", bufs=4) as opool,
            tc.tile_pool(name="mmpsum", bufs=4, space="PSUM") as mmpsum,
        ):
            xt = const.tile([128, NHALF], f32m)
            w1t = const.tile([128, NBLK * H], f32m)
            wht = const.tile([H, MPC * NH * H], f32m)
            whd = const.tile([H, MPC * 64], f32m)
            bh = const.tile([H, MPC * NH], f32)
            bhd = const.tile([128, 1], f32)

            # ordered so the first unit's operands land first: w1t + x
            # chunk 0 unblock layer-1 within a few microseconds, hidden
            # weights stream in per-model ahead of use, head weights last
            def dma_xt(nt):
                s = nt * CH
                nc.sync.dma_start(xt[:, s : s + CH], xt_d[:, s : s + CH])

            def dma_wht(m):
                s = m * NH * H
                nc.sync.dma_start(wht[:, s : s + NH * H], wht_d[:, s : s + NH * H])

            # group-0 L1 weights + first 512 x columns land first so the
            # very first matmul unblocks as early as possible
            nc.sync.dma_start(w1t[:, 0:H], w1t_d[:, 0:H])
            nc.sync.dma_start(xt[:, 0:MM_N], xt_d[:, 0:MM_N])
            nc.sync.dma_start(xt[:, MM_N:CH], xt_d[:, MM_N:CH])
            nc.sync.dma_start(w1t[:, H:], w1t_d[:, H:])
            nc.sync.dma_start(bh[:], bh_d[:])
            for m in range(4):
                dma_wht(m)
            dma_xt(1)
            for m in range(4, 12):
                dma_wht(m)
            dma_xt(2)
            dma_xt(3)
            for m in range(12, MPC):
                dma_wht(m)
            for nt in range(4, NCH):
                dma_xt(nt)
            nc.sync.dma_start(whd[:], whd_d[:])
            nc.sync.dma_start(bhd[:], bhd_d[:])

            def relu(dst, src, bias_ap, fd, force=None):
                # pick the engine that would FINISH this drain soonest given
                # both its queued load and the source MM's modeled completion
                # (constants fit from HW profile: ACT=(FD+311)/1.2,
                #  DVE=(FD+207)/0.96)
                ready = pe_state["t"]
                f_act = max(eng_load["act"], ready) + (fd + 311) / 1.2
                f_dve = max(eng_load["dve"], ready) + (fd + 207) / 0.96
                pick_act = f_act <= f_dve
                if force is not None:
                    pick_act = force == "act"
                if pick_act:
                    eng_load["act"] = f_act
                    if bias_ap is None:
                        nc.scalar.activation(dst, src, AF.Relu)
                    else:
                        nc.scalar.activation(dst, src, AF.Relu, bias=bias_ap)
                else:
                    eng_load["dve"] = f_dve
                    if bias_ap is None:
                        nc.vector.tensor_scalar_max(dst, src, 0.0)
                    else:
                        nc.vector.tensor_scalar(
                            dst, src, bias_ap, 0.0, ALU.add, ALU.max
                        )

            # hidden-pipeline groups of 5 (5x5=25, no 1-model straggler
            # group -- a 1-model unit is a PE production valley that starves
            # ACT/DVE at every chunk edge); L1 row-quadrants stay per-model
            groups = [list(range(b * 5, b * 5 + 5)) for b in range(5)]
            units = [(nt, bi) for nt in range(NCH) for bi in range(len(groups))]
            h_l1 = {}
            h_fin = {}  # (nt, m) -> final-layer h tile awaiting head matmuls

            def emit_l1_pair(nt, bi, p0, dve_bias=False):
                # 2-model row-tiled pair: each model gets its own contiguous
                # [128, CH] psum tile so its ReLU runs as one FD=CH op
                c0 = nt * CH
                grp = groups[bi]
                pair = grp[p0 : p0 + 2]
                for m in pair:
                    h_l1[(nt, m)] = hpool.tile([128, CH], f32m, tag="h", name="h")
                tiles = [
                    mmpsum.tile([128, CH], f32, tag="mm", name="l1ps")
                    for _ in pair
                ]
                for s in range(0, CH, MM_N):
                    for k, m in enumerate(pair):
                        # per-model w1t block/quadrant (host packs model m at
                        # block m//4, row-quadrant m%4); consecutive models
                        # always land in distinct quadrants -> pair overlaps
                        b, g = m // 4, m % 4
                        nc.tensor.matmul(
                            tiles[k][:, s : s + MM_N],
                            w1t[32 * g : 32 * g + DK, b * H : (b + 1) * H],
                            xt[32 * g : 32 * g + DK, c0 + s : c0 + s + MM_N],
                            start=True,
                            stop=True,
                            tile_position=(32 * g, 0),
                        )
                        pe_advance(MM_N / 2 + 64)
                for k, m in enumerate(pair):
                    # during a head streak DVE is the starving engine: pin
                    # the second relu of each mid-streak L1 pair to it
                    force = "dve" if (dve_bias and k == 1) else None
                    relu(h_l1[(nt, m)][:], tiles[k][:], None, CH, force=force)

            head_state = {}

            def emit_head_model(nt, m):
                """Head matmuls for one model, into the chunk's shared hp
                tile: even models stream through array cols 0-63 (psum
                partitions 0-63), odd through cols 64-127 (partitions
                64-127).  Accumulation groups interleave safely: HW
                has_written clearing is per written region (verified)."""
                if "hp" not in head_state:
                    head_state["hp"] = mmpsum.tile([128, CH], f32, tag="mm", name="hp")
                hp = head_state["hp"]
                g = m % 2
                lhshd = whd[:, m * 64 : (m + 1) * 64]
                hf = h_fin.pop((nt, m))
                for s in range(0, CH, MM_N):
                    nc.tensor.matmul(
                        hp[64 * g : 64 * g + 64, s : s + MM_N],
                        lhshd,
                        hf[:, s : s + MM_N],
                        start=(m <= 1),
                        stop=(m >= MPC - 2),
                        tile_position=(0, 64 * g),
                        skip_group_check=True,
                    )
                    pe_advance(MM_N / 2 + 50)

            def emit_head_epilogue(nt):
                """mu (DVE bias-add) / sigma (ACT exp) + DMA out."""
                c0 = nt * CH
                hp = head_state.pop("hp")
                mu_t = opool.tile([128, CH], f32, tag="mu")
                sig_t = opool.tile([128, CH], f32, tag="sig")
                # each op's DMA issues right after it (not batched at the
                # end) so outputs drain and opool tiles free ~1-2us earlier
                nc.vector.tensor_scalar_add(
                    mu_t[0:NEV, :], hp[0:NEV, :], bhd[0:NEV, :]
                )
                nc.sync.dma_start(mu_d[0:NEV, c0 : c0 + CH], mu_t[0:NEV, :])
                nc.scalar.activation(
                    sig_t[32 : 32 + NEV, :], hp[32 : 32 + NEV, :], AF.Exp,
                    bias=bhd[32 : 32 + NEV, :],
                )
                nc.sync.dma_start(
                    sig_d[0:NEV, c0 : c0 + CH], sig_t[32 : 32 + NEV, :]
                )
                nc.vector.tensor_scalar_add(
                    mu_t[64 : 64 + NOD, :], hp[64 : 64 + NOD, :],
                    bhd[64 : 64 + NOD, :],
                )
                nc.sync.dma_start(
                    mu_d[NEV:MPC, c0 : c0 + CH], mu_t[64 : 64 + NOD, :]
                )
                eng_load["dve"] += 2 * (CH + 207) / 0.96
                nc.scalar.activation(
                    sig_t[96 : 96 + NOD, :], hp[96 : 96 + NOD, :], AF.Exp,
                    bias=bhd[96 : 96 + NOD, :],
                )
                nc.sync.dma_start(
                    sig_d[NEV:MPC, c0 : c0 + CH], sig_t[96 : 96 + NOD, :]
                )
                eng_load["act"] += 2 * (CH + 311) / 1.2

            l1_emitted = set()

            def maybe_emit_l1_pair(uidx, p0, dve_bias=False):
                if uidx < len(units) and (uidx, p0) not in l1_emitted:
                    nt2, bi2 = units[uidx]
                    if p0 < len(groups[bi2]):
                        l1_emitted.add((uidx, p0))
                        emit_l1_pair(nt2, bi2, p0, dve_bias=dve_bias)

            def maybe_emit_l1(uidx):
                if uidx < len(units):
                    for p0 in range(0, len(groups[units[uidx][1]]), 2):
                        maybe_emit_l1_pair(uidx, p0)

            maybe_emit_l1(0)
            LAST = NCH - 1

            def emit_head_streak(ntq, l1_uidx=None):
                # Interleave the next unit's two L1 matmul pairs into the
                # streak so the PE keeps producing relu-able psum tiles
                # mid-streak: each engine dry-spell shrinks to roughly the
                # backlog the engines can cover.  Epilogue stays strictly
                # after ALL head MMs (overlapped PSUM reads of the hp tile
                # were measured race-prone on HW -- do not reorder).
                segs = [(0, 4), (4, 8), (8, 12), (12, 16), (16, 20), (20, MPC)]
                l1_feed = [(0, 0), (0, 2), (0, 4), (1, 0), (1, 2), (1, 4)]
                for (m0, m1), (du, p0) in zip(segs, l1_feed):
                    if l1_uidx is not None:
                        maybe_emit_l1_pair(l1_uidx + du, p0, dve_bias=True)
                    for mq in range(m0, m1):
                        emit_head_model(ntq, mq)
                emit_head_epilogue(ntq)

            for u, (nt, bi) in enumerate(units):
                grp = groups[bi]
                last_unit = u == len(units) - 1
                if last_unit:
                    # the final chunk's heads for models finished by earlier
                    # units run interleaved with this unit so only the last
                    # group's heads + epilogue remain as tail
                    for mq in range(0, 12):
                        emit_head_model(LAST, mq)
                hcur = {m: h_l1.pop((nt, m)) for m in grp}
                # hidden layers, interleaved across the group
                for i in range(NH):
                    for m in grp:
                        ps = mmpsum.tile([128, CH], f32, tag="mm")
                        lhsh = wht[:, (m * NH + i) * H : (m * NH + i + 1) * H]
                        for s in range(0, CH, MM_N):
                            nc.tensor.matmul(
                                ps[:, s : s + MM_N],
                                lhsh,
                                hcur[m][:, s : s + MM_N],
                                start=True,
                                stop=True,
                            )
                            pe_advance(MM_N + 6)
                        hn = hpool.tile([128, CH], f32m, tag="h")
                        bias_ap = bh[:, m * NH + i : m * NH + i + 1]
                        relu(hn[:], ps[:], bias_ap, CH)
                        hcur[m] = hn
                    if i == NH - 2:
                        if bi == 2 and nt >= 1:
                            # previous chunk's head streak with the next
                            # unit's L1 pairs interleaved mid-streak so
                            # ACT/DVE keep getting fresh relu work
                            emit_head_streak(nt - 1, l1_uidx=u + 1)
                        else:
                            # prefetch the next unit's layer-1
                            maybe_emit_l1(u + 1)
                    if last_unit and i == 1:
                        for mq in range(12, 20):
                            emit_head_model(LAST, mq)
                for m in grp:
                    h_fin[(nt, m)] = hcur[m]
            for mq in range(20, MPC):
                emit_head_model(LAST, mq)
            emit_head_epilogue(LAST)

    _dedupe_ldweights(nc, mybir)
    nc.compile()
    return nc


def _dedupe_ldweights(nc, mybir):
    """Delete LDWEIGHTS whose exact weights are already resident in the same
    PE-array region (the Tile lowering re-emits one per matmul).  Weights
    persist in the array across matmuls, so back-to-back matmuls on the same
    stationary operand only need the first load.  Region tracking handles
    row/col-tiled partial loads (an overlapping load invalidates)."""
    removed = 0
    for blk in nc.main_func.blocks:
        loaded: dict = {}
        out = []
        for inst in blk.instructions:
            if isinstance(inst, mybir.InstLdweights):
                w = inst.ins[0]
                key = (
                    getattr(w, "memref", None),
                    w.offset,
                    str(w.ap),
                    str(w.dtype),
                    inst.tile_position,
                    inst.tile_size,
                    inst.perf_mode,
                    inst.is_transpose,
                )
                tp = inst.tile_position or (0, 0)
                ts = inst.tile_size or (128, 128)
                region = (tp[0], tp[0] + ts[0], tp[1], tp[1] + ts[1])
                si = inst.sync_info
                no_sync = si is None or (not si.on_wait and not si.on_update)
                if no_sync and loaded.get(region) == key:
                    removed += 1
                    continue
                for r in list(loaded):
                    if not (
                        r[1] <= region[0]
                        or region[1] <= r[0]
                        or r[3] <= region[2]
                        or region[3] <= r[2]
                    ):
                        del loaded[r]
                loaded[region] = key
            out.append(inst)
        blk.instructions[:] = out
    return removed


def _get_module():
    if "nc" not in _CACHE:
        _CACHE["nc"] = _build_module()
    return _CACHE["nc"]


def _mm_np_dtype():
    mmdt = os.environ.get("KERNEL_MM_DTYPE", "bf16")
    if mmdt == "bf16":
        import ml_dtypes

        return ml_dtypes.bfloat16
    if mmdt == "fp16":
        return np.float16
    return np.float32


def _shard_inputs(x, W1, b1, Wh, bh, Wmu, bmu, Wsig, bsig):
    """Build the per-core input maps (host-side layout prep)."""
    NBLK = (MPC + 3) // 4
    mdt = _mm_np_dtype()
    in_maps = []
    for c in range(NCORES):
        mb, half = c % NB, c // NB
        ms = slice(MPC * mb, MPC * (mb + 1))
        xh = x[NHALF * half : NHALF * (half + 1), :]  # [8192, 16]
        xtr = np.ascontiguousarray(xh.T)  # [16, 8192]
        xt_full = np.zeros((128, NHALF), dtype=np.float32)
        for rep in range(4):  # replicas at partition 0/32/64/96 for row tiling
            xt_full[32 * rep : 32 * rep + D, :] = xtr
            xt_full[32 * rep + D, :] = 1.0  # constant row for folded L1 bias

        w1 = W1[ms]  # [25, 128, 16]
        b1c = b1[ms]  # [25, 128]
        w1t = np.zeros((128, NBLK * H), dtype=np.float32)
        for m in range(MPC):
            b, g = m // 4, m % 4
            w1t[32 * g : 32 * g + D, b * H : (b + 1) * H] = w1[m].T
            w1t[32 * g + D, b * H : (b + 1) * H] = b1c[m]  # folded bias row

        wh = Wh[ms]  # [25, 4, 128, 128] (out, in)
        wht = np.ascontiguousarray(
            wh.transpose(3, 0, 1, 2).reshape(H, MPC * NH * H)
        )  # [h_in, (m, i, h_out)]

        # head weights, col-tiled 2x: even models -> array cols 0-63, odd ->
        # cols 64-127; within the 64-col block: col k=m//2 = Wmu, 32+k = Wsig
        whd = np.zeros((H, MPC * 64), dtype=np.float32)
        for m in range(MPC):
            base, k = m * 64, m // 2
            whd[:, base + k] = Wmu[ms][m, 0, :]
            whd[:, base + 32 + k] = Wsig[ms][m, 0, :]

        bhp = np.ascontiguousarray(
            bh[ms].transpose(2, 0, 1).reshape(H, MPC * NH)
        )  # [128, (m, i)]
        bhdp = np.zeros((128, 1), dtype=np.float32)
        bhdp[0:NEV, 0] = bmu[ms][0::2, 0]
        bhdp[32 : 32 + NEV, 0] = bsig[ms][0::2, 0]
        bhdp[64 : 64 + NOD, 0] = bmu[ms][1::2, 0]
        bhdp[96 : 96 + NOD, 0] = bsig[ms][1::2, 0]

        in_maps.append(
            {
                "xt": xt_full.astype(mdt),
                "w1t": w1t.astype(mdt),
                "wht": wht.astype(mdt),
                "whd": whd.astype(mdt),
                "bh": bhp,
                "bhd": bhdp,
            }
        )
    return in_maps


def _run(in_maps, trace=False):
    from concourse.bass_utils import run_bass_kernel_spmd

    nc = _get_module()
    return run_bass_kernel_spmd(
        nc, in_maps, list(range(NCORES)), trace=trace
    )


def kernel(x, W1, b1, Wh, bh, Wmu, bmu, Wsig, bsig):
    args = [
        np.ascontiguousarray(np.asarray(a, dtype=np.float32))
        for a in (x, W1, b1, Wh, bh, Wmu, bmu, Wsig, bsig)
    ]
    in_maps = _shard_inputs(*args)
    res = _run(in_maps, trace=bool(int(os.environ.get("KERNEL_TRACE", "0"))))
    _CACHE["last_results"] = res

    mu = np.empty((M, N), dtype=np.float32)
    sig = np.empty((M, N), dtype=np.float32)
    ev = np.arange(0, MPC, 2)  # storage rows 0..12 hold even models
    od = np.arange(1, MPC, 2)  # storage rows 13..24 hold odd models
    for c in range(NCORES):
        mb, half = c % NB, c // NB
        m0 = MPC * mb
        ns = slice(NHALF * half, NHALF * (half + 1))
        r = res.results[c]
        mu[m0 + ev, ns] = r["mu"][0:NEV]
        mu[m0 + od, ns] = r["mu"][NEV:MPC]
        sig[m0 + ev, ns] = r["sig"][0:NEV]
        sig[m0 + od, ns] = r["sig"][NEV:MPC]
    return (mu.reshape(M, N, O), sig.reshape(M, N, O))
